# revision 23
# baseline (speedup 1.0000x reference)
"""Trainium2 Bass kernel for nn_AffineContour (gnn_message_passing).

Computation (reference):
    x_even = x[even_indices]                        # (65536,)
    u = relu(relu(x_even @ u_W0 + u_b0) @ u_W1 + u_b1)
    v = relu(relu(x_even @ v_W0 + v_b0) @ v_W1 + v_b1)
    u_s = (u @ us_W + us_b)[0];  u_t = (v @ ut_W + ut_b)[0]
    y = x + 1j * scatter(odd_indices, u_s * x[odd_indices] + u_t)

Memory-bound by the two (65536, 512) fp32 first-layer weights (268 MB).

Strategy (per core j of 8):
- Shard the GEMV contraction dim row-wise: 8192 rows of each W0 per core
  (33.5 MB), streamed in 2 MB tiles on both HWDGE rings, consumed by
  float32r matmuls (full-rate fp32 at N=512) accumulating [1,512]
  partials in PSUM.  The layer-1 bias enters the same accumulation as a
  K=1 matmul against b0/8 (8 cores sum to b0 under the collective).
- A dummy warmup collective fired at kernel start absorbs ncfw's
  ~30-50us first-call cost; one combined 4 KB/rank AllGather after both
  GEMVs then runs at the ~14us floor.
- Rank reduction of gathered partials is four K=8 matmuls against
  one-hot mask columns, which simultaneously transposes [8x512] into
  the [128,4] partition-major layout layer 2 needs; relu on the way out.
- Layer 2 + heads are replicated on every core (1 MB weights, K=1 bias
  row matmuls, fused max0*mul+reduce via scalar_tensor_tensor), the
  scalars are broadcast via a K=1 matmul, and each core emits its 1/8
  slice of u_s * x_odd + u_t.  The host assembles the complex output.
"""

import threading

import numpy as np

import concourse.bacc as bacc
import concourse.bass as bass
import concourse.mybir as mybir
import concourse.tile as tile
from concourse.bass_utils import run_bass_kernel_spmd

V = 131072
VH = 65536
F = 512
NCORES = 8
QPC = 64          # contraction chunks (of 128 rows) per core
G = 8             # chunks per weight DMA group
NG = QPC // G     # weight DMA groups per matrix per core
F32 = mybir.dt.float32
F32R = mybir.dt.float32r

_lock = threading.Lock()
_cache = {}


def build_nc():
    nc = bacc.Bacc(
        "TRN2",
        debug=False,
        enable_asserts=False,
        target_bir_lowering=False,
        num_devices=NCORES,
    )

    w0 = nc.dram_tensor("w0", [2, NG, 128, G * F], F32R, kind="ExternalInput")
    w1 = nc.dram_tensor("w1", [2, 128, 4 * F], F32R, kind="ExternalInput")
    xe_t = nc.dram_tensor("xe_t", [128, QPC], F32R, kind="ExternalInput")
    xo_t = nc.dram_tensor("xo_t", [128, 64], F32, kind="ExternalInput")
    b0r = nc.dram_tensor("b0r", [1, 2 * F], F32R, kind="ExternalInput")  # b0/8 rows
    b1r = nc.dram_tensor("b1r", [1, 2 * F], F32R, kind="ExternalInput")
    hw = nc.dram_tensor("hw", [1, 2 * F], F32, kind="ExternalInput")
    hb = nc.dram_tensor("hb", [1, 2], F32, kind="ExternalInput")
    ones8_d = nc.dram_tensor("ones8", [8, 1], F32R, kind="ExternalInput")
    rmask_d = nc.dram_tensor("rmask", [8, 16], F32R, kind="ExternalInput")

    vals_out = nc.dram_tensor("vals_out", [VH // NCORES], F32, kind="ExternalOutput")

    cc_in = nc.dram_tensor("cc_in", [1, 2 * F], F32R)
    cc_out = nc.dram_tensor("cc_out", [NCORES, 2 * F], F32R, addr_space="Shared")
    warm_in = nc.dram_tensor("warm_in", [1, 16], F32)
    warm_out = nc.dram_tensor("warm_out", [NCORES, 16], F32, addr_space="Shared")

    with tile.TileContext(nc) as tc:
        with (
            tc.tile_pool(name="wpool", bufs=5) as wpool,
            tc.tile_pool(name="spool", bufs=1) as spool,
            tc.tile_pool(name="psum", bufs=1, space="PSUM") as psum,
        ):
            # L1-critical small inputs lead the sync ring (ahead of weights)
            xe = spool.tile([128, QPC], F32R)
            nc.sync.dma_start(xe[:], xe_t[:])
            b0_sb = spool.tile([1, 2 * F], F32R)
            nc.sync.dma_start(b0_sb[:], b0r[:])
            xo = spool.tile([128, 64], F32)
            nc.gpsimd.dma_start(xo[:], xo_t[:])
            b1_sb = spool.tile([1, 2 * F], F32R)
            nc.gpsimd.dma_start(b1_sb[:], b1r[:])
            hw_sb = spool.tile([1, 2 * F], F32)
            nc.gpsimd.dma_start(hw_sb[:], hw[:])
            hb_sb = spool.tile([1, 2], F32)
            nc.gpsimd.dma_start(hb_sb[:], hb[:])
            w1_sb = spool.tile([128, 2 * 4 * F], F32R)
            nc.gpsimd.dma_start(w1_sb[:, 0 : 4 * F], w1.ap()[0])
            nc.gpsimd.dma_start(w1_sb[:, 4 * F : 8 * F], w1.ap()[1])
            ones = spool.tile([1, 128], F32)
            nc.vector.memset(ones[:], 1.0)
            ones8 = spool.tile([8, 1], F32R)
            nc.sync.dma_start(ones8[:], ones8_d[:])
            rmask = spool.tile([8, 16], F32R)
            nc.sync.dma_start(rmask[:], rmask_d[:])
            one1 = ones8

            dma_engines = [nc.sync, nc.scalar]

            # tiny dummy collective fired immediately: absorbs ncfw's
            # 30-50us first-call warmup so the real AllGather runs at floor
            nc.gpsimd.collective_compute(
                "AllGather",
                mybir.AluOpType.bypass,
                replica_groups=[list(range(NCORES))],
                ins=[warm_in[:]],
                outs=[warm_out[:]],
            )

            # ---- layer 1: row-sharded GEMV per MLP (u first, then v) ----
            # each MLP: bias/8 enters as a K=1 matmul, then 64 K=128 chunks
            psums1 = [psum.tile([1, F], F32, name=f"psum1_{m}") for m in range(2)]
            partials = [
                spool.tile([1, F], F32R, name=f"partial_{m}") for m in range(2)
            ]
            for m in range(2):
                for g in range(NG):
                    wt = wpool.tile([128, G * F], F32R)
                    if m == 1 and g == NG - 1:
                        # last group: quarter-DMAs so the final matmuls chase
                        # the stream tail instead of waiting for the full 2 MB
                        for q in range(4):
                            dma_engines[q % 2].dma_start(
                                wt[:, 1024 * q : 1024 * (q + 1)],
                                w0.ap()[m, g][:, 1024 * q : 1024 * (q + 1)],
                            )
                    else:
                        dma_engines[g % 2].dma_start(wt[:], w0.ap()[m, g])
                    for t in range(G):
                        nc.tensor.matmul(
                            psums1[m][:],
                            xe[:, g * G + t : g * G + t + 1],
                            wt[:, t * F : (t + 1) * F],
                            start=(g == 0 and t == 0),
                            stop=False,
                        )
                nc.tensor.matmul(
                    psums1[m][:],
                    one1[0:1, :],
                    b0_sb[:, m * F : (m + 1) * F],
                    start=False,
                    stop=True,
                )
                # ship this MLP's partial into its half of the cc payload
                # (u via SWDGE to keep the weight rings unblocked; v on the
                # sync ring, which is past all weight DMAs by then)
                nc.vector.tensor_copy(partials[m][:], psums1[m][:])
                ship = nc.gpsimd if m == 0 else nc.sync
                ship.dma_start(cc_in.ap()[:, m * F : (m + 1) * F], partials[m][:])

            # single AllGather of both partials (4 KB/rank), fired once v is done
            nc.gpsimd.collective_compute(
                "AllGather",
                mybir.AluOpType.bypass,
                replica_groups=[list(range(NCORES))],
                ins=[cc_in[:]],
                outs=[cc_out[:]],
            )
            ag = spool.tile([8, 2 * F], F32R)
            nc.sync.dma_start(ag[:, 0:F], cc_out.ap()[:, 0:F])
            nc.scalar.dma_start(ag[:, F : 2 * F], cc_out.ap()[:, F : 2 * F])

            # ---- tail: reduce ranks (PE transpose-sum), relu, layer 2 ----
            st = spool.tile([1, 2], F32)
            for m in range(2):
                # psum_t[p, r] = sum_k ag[k, m*512 + r*128+p]: reduce + transpose
                psum_t = psum.tile([128, 4], F32, name=f"psum_t_{m}")
                for r in range(4):
                    nc.tensor.matmul(
                        psum_t[:],
                        ag[:, m * F + 128 * r : m * F + 128 * (r + 1)],
                        rmask[:, 4 * r : 4 * (r + 1)],
                        start=(r == 0),
                        stop=(r == 3),
                    )
                uvr = spool.tile([128, 4], F32R, name=f"uvr_{m}")
                nc.vector.tensor_relu(uvr[:], psum_t[:])

                psum2 = psum.tile([1, F], F32, name=f"psum2_{m}")
                nc.tensor.matmul(
                    psum2[:],
                    one1[0:1, :],
                    b1_sb[:, m * F : (m + 1) * F],
                    start=True,
                    stop=False,
                )
                for r in range(4):
                    nc.tensor.matmul(
                        psum2[:],
                        uvr[:, r : r + 1],
                        w1_sb[:, (4 * m + r) * F : (4 * m + r + 1) * F],
                        start=False,
                        stop=(r == 3),
                    )
                # head: st[m] = sum(max(psum2, 0) * hw[m])
                junk = spool.tile([1, F], F32, name=f"junk_{m}")
                nc.vector.scalar_tensor_tensor(
                    junk[:],
                    psum2[:],
                    0.0,
                    hw_sb[:, m * F : (m + 1) * F],
                    op0=mybir.AluOpType.max,
                    op1=mybir.AluOpType.mult,
                    accum_out=st[:, m : m + 1],
                )
            st2 = spool.tile([1, 2], F32)
            nc.vector.tensor_add(st2[:], st[:], hb_sb[:])

            # broadcast (u_s, u_t) to all 128 partitions via a K=1 matmul
            psum_bc = psum.tile([128, 2], F32)
            nc.tensor.matmul(psum_bc[:], ones[:], st2[:], start=True, stop=True)
            st_T = spool.tile([128, 2], F32)
            nc.vector.tensor_copy(st_T[:], psum_bc[:])

            # vals = u_s * x_odd + u_t for this core's slice
            vals = spool.tile([128, 64], F32)
            nc.vector.tensor_scalar(
                vals[:], xo[:], st_T[:, 0:1], st_T[:, 1:2],
                op0=mybir.AluOpType.mult, op1=mybir.AluOpType.add,
            )
            nc.sync.dma_start(
                vals_out.ap().rearrange("(p t) -> p t", p=128), vals[:]
            )

    nc.compile()
    return nc


def _prep_w0(W, j):
    # chunk (g, t) covers rows r = p*512 + 64*j + 8*g + t
    A = W.reshape(128, 512, F)[:, QPC * j : QPC * (j + 1), :]
    A = A.reshape(128, NG, G, F).transpose(1, 0, 2, 3)
    return np.ascontiguousarray(A).reshape(NG, 128, G * F)


def _rmask():
    m = np.zeros((8, 16), dtype=np.float32)
    for r in range(4):
        m[:, 4 * r + r] = 1.0
    return m


def _prep_w1(W):
    return np.ascontiguousarray(W.reshape(4, 128, F).transpose(1, 0, 2)).reshape(
        128, 4 * F
    )


def make_in_maps(
    x, u_W0, u_b0, u_W1, u_b1, v_W0, v_b0, v_W1, v_b1,
    us_W, us_b, ut_W, ut_b, even_indices, odd_indices,
):
    x = np.asarray(x, dtype=np.float32)
    xe = x[np.asarray(even_indices)].astype(np.float32)
    xo = x[np.asarray(odd_indices)].astype(np.float32)
    xe_m = xe.reshape(128, 512)

    b0r = (np.concatenate([u_b0, v_b0]).astype(np.float32) / 8.0)[None, :]
    b1r = np.concatenate([u_b1, v_b1]).astype(np.float32)[None, :]
    hw = np.concatenate([us_W[:, 0], ut_W[:, 0]]).astype(np.float32)[None, :]
    hb = np.concatenate([us_b, ut_b]).astype(np.float32)[None, :]
    w1 = np.stack([_prep_w1(np.asarray(u_W1, np.float32)),
                   _prep_w1(np.asarray(v_W1, np.float32))])

    rpc = VH // NCORES
    in_maps = []
    for j in range(NCORES):
        in_maps.append(
            {
                "w0": np.stack(
                    [_prep_w0(u_W0, j), _prep_w0(v_W0, j)]
                ).astype(np.float32, copy=False),
                "w1": w1,
                "xe_t": np.ascontiguousarray(xe_m[:, QPC * j : QPC * (j + 1)]),
                "xo_t": xo[rpc * j : rpc * (j + 1)].reshape(128, 64),
                "b0r": b0r,
                "b1r": b1r,
                "hw": hw,
                "hb": hb,
                "ones8": np.ones((8, 1), dtype=np.float32),
                "rmask": _rmask(),
            }
        )
    return in_maps


def kernel(
    x, u_W0, u_b0, u_W1, u_b1, v_W0, v_b0, v_W1, v_b1,
    us_W, us_b, ut_W, ut_b, even_indices, odd_indices,
):
    x = np.asarray(x, dtype=np.float32)
    even_indices = np.asarray(even_indices)
    odd_indices = np.asarray(odd_indices)

    with _lock:
        if "nc" not in _cache:
            _cache["nc"] = build_nc()
    nc = _cache["nc"]

    in_maps = make_in_maps(
        x, u_W0, u_b0, u_W1, u_b1, v_W0, v_b0, v_W1, v_b1,
        us_W, us_b, ut_W, ut_b, even_indices, odd_indices,
    )

    res = run_bass_kernel_spmd(nc, in_maps, core_ids=list(range(NCORES)))
    vals = np.concatenate([res.results[j]["vals_out"] for j in range(NCORES)])

    imag = np.zeros(V, dtype=np.float32)
    np.add.at(imag, odd_indices, vals)
    y = np.empty(V, dtype=np.complex64)
    y.real = x
    y.imag = imag
    return y
